# revision 8
# baseline (speedup 1.0000x reference)
"""Fused linear + cross-entropy loss (chunked logsumexp) on 8 NeuronCores.

Strategy: tensor-parallel over vocab. Each core holds a 4000-row shard of
head_weight, computes logits = h @ W_c^T for all 4096 tokens (fp8e4m3
DoubleRow matmuls by default; bf16 fallback when head_bias is nonzero),
and reduces sum(exp(logit)) per token on the ACT engine (exp with
accum_out; the pre-exp rescale for the fp8 weight scaling rides the
ACT's free scale operand). The target-logit term is a per-token dot
h[t] . W[label_t] computed on the DVE in bf16 from host-gathered rows
(data-parallel over tokens). The host does only glue: transpose/cast/
shard, the final log over 4096 values, and the weighted mean.
"""

import numpy as np
import ml_dtypes

T = 4096
D = 1024
V = 32000
NCORES = 8
VSH = V // NCORES        # 4000 vocab rows per core
CPH = VSH // 2           # 2000 vocab cols per half
TT = T // 128            # 32 token tiles
TBC = 512                # tokens per resident ht block
NTB = T // TBC           # 8 ht col blocks
TLOC = T // NCORES       # 512 tokens per core for the target dot
JT = TLOC // 128         # 4 local token tiles

W_SCALE = 32.0           # fp8 path: W is scaled by this before casting
USE_FP8 = True

_CACHE = {}


def _chunks(cols):
    """Split cols into matmul free-dim chunks (<=512, 16-aligned)."""
    out = []
    while cols > 0:
        c = min(cols, 512)
        out.append(c)
        cols -= c
    assert all(c % 16 == 0 for c in out)
    return out


def _build(kt, mode, t=T, vsh=VSH, jt=JT, d=D, do_compile=True):
    """Build+compile the SPMD Bass program.

    kt: number of 128-deep k tiles (8, or 9 when a nonzero head_bias is
        folded in as an extra contraction row).
    mode: "bf16" (plain matmuls) or "fp8dr" (fp8e4m3 DoubleRow, kt even).
    """
    import concourse.bass as bass
    import concourse.mybir as mybir
    import concourse.tile as tile
    from concourse import bacc

    f32 = mybir.dt.float32
    bf16 = mybir.dt.bfloat16
    fp8 = mybir.dt.float8e4
    AF = mybir.ActivationFunctionType
    ALU = mybir.AluOpType

    fp8dr = mode == "fp8dr"
    mdt = fp8 if fp8dr else bf16
    act_scale = (1.0 / W_SCALE) if fp8dr else 1.0
    if fp8dr:
        assert kt % 2 == 0
    nk = kt // 2 if fp8dr else kt   # matmul contraction steps

    tt = t // 128
    tb = min(TBC // 128, tt)   # token tiles per ht block
    ntb = tt // tb
    cph = vsh // 2
    CH = _chunks(cph)          # e.g. [512, 512, 512, 464]
    nch = len(CH)
    nsteps = 2 * tt

    nc = bacc.Bacc("TRN2", target_bir_lowering=False, debug=False)

    ht_d = nc.dram_tensor("ht", [ntb, 128, kt, tb * 128], mdt,
                          kind="ExternalInput")
    w_d = {}
    for half in range(2):
        for ci, w in enumerate(CH):
            w_d[half, ci] = nc.dram_tensor(
                f"w_{half}_{ci}", [128, kt, w], mdt, kind="ExternalInput"
            )
    hrow_d = nc.dram_tensor("hrow", [jt, 128, d], bf16, kind="ExternalInput")
    wg_d = nc.dram_tensor("wg", [jt, 128, d], bf16, kind="ExternalInput")
    hsums_d = nc.dram_tensor("hsums", [128, nsteps], f32,
                             kind="ExternalOutput")
    tgt_d = nc.dram_tensor("tgt", [128, jt], f32, kind="ExternalOutput")

    with tile.TileContext(nc) as tc:
        with (
            tc.tile_pool(name="w", bufs=1) as wpool,
            tc.tile_pool(name="h", bufs=1) as hpool,
            tc.tile_pool(name="dot", bufs=1) as dpool,
            tc.tile_pool(name="stat", bufs=1) as spool,
            tc.tile_pool(name="sink", bufs=2) as kpool,
            tc.tile_pool(name="ps", bufs=2, space="PSUM") as ppool,
        ):
            wt = {}
            ht = [None] * ntb

            def load_w(half, ci, split=1):
                w = CH[ci]
                tl = wpool.tile([128, kt, w], mdt, tag=f"w{half}_{ci}")
                if split == 1:
                    nc.sync.dma_start(tl[:], w_d[half, ci][:])
                else:
                    kh = -(-kt // split)
                    for s in range(split):
                        k0, k1 = s * kh, min((s + 1) * kh, kt)
                        nc.sync.dma_start(
                            tl[:, k0:k1, :],
                            w_d[half, ci][:, k0:k1, :],
                        )
                wt[half, ci] = tl

            def load_h(b, split=1):
                tl = hpool.tile([128, kt, tb * 128], mdt, tag=f"h{b}")
                if split == 1:
                    nc.sync.dma_start(tl[:], ht_d[b])
                else:
                    cw = tb * 128 // split
                    for s in range(split):
                        nc.sync.dma_start(
                            tl[:, :, s * cw:(s + 1) * cw],
                            ht_d[b, :, :, s * cw:(s + 1) * cw],
                        )
                ht[b] = tl

            # First-needed data first; compute starts as pieces land.
            load_w(0, 0, split=2)
            load_h(0, split=2)
            for ci in range(1, nch):
                load_w(0, ci)
            for b in range(1, ntb):
                load_h(b)
            for ci in range(nch):
                load_w(1, ci)

            # PE warmup during the DMA wait: junk matmuls from a memset
            # tile keep the HAM activity window busy so real matmuls run
            # at full clock. Writes the first ps slot; real groups clear
            # the bank with start=True before use.
            warm = kpool.tile([128, 256], mdt, tag="warm")
            nc.gpsimd.memset(warm[:], 0.0)
            ps_w = ppool.tile([128, nch, 512], f32, tag="ps")
            for _ in range(44):
                nc.tensor.matmul(
                    ps_w[:, 0, 0:128], warm[:, 0:128], warm[:, 128:256],
                    start=True, stop=True,
                )

            hsums = spool.tile([128, nsteps], f32, tag="hsums")

            # Target dot: tgt[p, j] = sum_d hrow[j,p,d] * wg[j,p,d]  (DVE)
            tgt_sb = spool.tile([128, jt], f32, tag="tgt")
            for j in range(jt):
                hr = dpool.tile([128, d], bf16, tag=f"hr{j}")
                wr = dpool.tile([128, d], bf16, tag=f"wr{j}")
                nc.sync.dma_start(hr[:], hrow_d[j])
                nc.sync.dma_start(wr[:], wg_d[j])
                dsink = kpool.tile([128, d], f32, tag="dsink")
                nc.vector.tensor_tensor(dsink[:], hr[:], wr[:], ALU.mult)
                nc.vector.tensor_reduce(
                    tgt_sb[:, j:j + 1],
                    dsink[:],
                    axis=mybir.AxisListType.X,
                    op=ALU.add,
                )
            nc.sync.dma_start(tgt_d[:], tgt_sb[:])

            def mm(ps, hblk, mlo, half, ki, ci):
                rhs_t = wt[half, ci]
                w = CH[ci]
                if fp8dr:
                    nc.tensor.matmul(
                        ps[:, ci, 0:w],
                        hblk[:, 2 * ki:2 * ki + 2, mlo:mlo + 128],
                        rhs_t[:, 2 * ki:2 * ki + 2, :],
                        start=(ki == 0),
                        stop=(ki == nk - 1),
                        perf_mode=mybir.MatmulPerfMode.DoubleRow,
                    )
                else:
                    nc.tensor.matmul(
                        ps[:, ci, 0:w],
                        hblk[:, ki, mlo:mlo + 128],
                        rhs_t[:, ki, :],
                        start=(ki == 0),
                        stop=(ki == nk - 1),
                    )

            def step(half, t_i, order):
                s = half * tt + t_i
                hblk = ht[t_i // tb]
                mlo = (t_i % tb) * 128
                ps = ppool.tile([128, nch, 512], f32, tag="ps")
                if order == "k":
                    for ki in range(nk):
                        for ci in range(nch):
                            mm(ps, hblk, mlo, half, ki, ci)
                else:
                    for ci in range(nch):
                        for ki in range(nk):
                            mm(ps, hblk, mlo, half, ki, ci)
                # One ACT over all banks. Unwritten PSUM cols (the tail of
                # the last bank) read as zero after start=True cleared the
                # bank, contributing exp(0)=1 each; host subtracts them.
                esink = kpool.tile([128, nch * 512], bf16, tag="esink")
                nc.scalar.activation(
                    esink[:],
                    ps[:, :, :],
                    AF.Exp,
                    scale=act_scale,
                    accum_out=hsums[:, s:s + 1],
                )

            for t_i in range(tt):
                step(0, t_i, "c" if t_i < 4 else "k")
            for t_i in range(tt):
                step(1, t_i, "k")

            nc.sync.dma_start(hsums_d[:], hsums[:])

    if do_compile:
        nc.compile()
    return nc


def _get_nc(kt, mode):
    key = (kt, mode)
    if key not in _CACHE:
        _CACHE[key] = _build(kt, mode)
    return _CACHE[key]


def kernel(hidden_states, head_weight, head_bias, labels, loss_weight):
    from concourse.bass_utils import run_bass_kernel_spmd

    bf16 = ml_dtypes.bfloat16
    fp8 = ml_dtypes.float8_e4m3
    h = np.ascontiguousarray(np.asarray(hidden_states, dtype=np.float32))
    W = np.ascontiguousarray(np.asarray(head_weight, dtype=np.float32))
    b = np.asarray(head_bias, dtype=np.float32)
    lab = np.asarray(labels).astype(np.int64)
    lw = np.asarray(loss_weight, dtype=np.float32)

    use_bias = bool(np.any(b))
    mode = "fp8dr" if (USE_FP8 and not use_bias) else "bf16"
    mdt = fp8 if mode == "fp8dr" else bf16
    wscale = W_SCALE if mode == "fp8dr" else 1.0
    kt = 9 if use_bias else 8
    nc = _get_nc(kt, mode)
    CH = _chunks(CPH)

    # hT[k, p, t] = h[t, k*128+p]; ht blocks [ntb, 128, kt, TBC].
    hT = np.zeros((kt, 128, T), dtype=np.float32)
    hT[:8] = np.ascontiguousarray(h.T).reshape(8, 128, T)
    if use_bias:
        hT[8, 0, :] = 1.0
    ht_blocks = np.ascontiguousarray(
        hT.reshape(kt, 128, NTB, TBC).transpose(2, 1, 0, 3).astype(mdt)
    )

    Wg = W[lab]                     # [T, D] gathered target rows
    tgt_bias = b[lab]               # [T]

    in_maps = []
    for c in range(NCORES):
        Wc = np.ascontiguousarray(W[c * VSH:(c + 1) * VSH].T) * wscale
        # wT[k, p, v] = Wc.T[k*128+p, v] (scaled)
        wT = np.zeros((kt, 128, VSH), dtype=np.float32)
        wT[:8] = Wc.reshape(8, 128, VSH)
        if use_bias:
            wT[8, 0, :] = b[c * VSH:(c + 1) * VSH]
        m = {}
        off = 0
        for half in range(2):
            for ci, w in enumerate(CH):
                blk = wT[:, :, off:off + w].transpose(1, 0, 2).astype(mdt)
                m[f"w_{half}_{ci}"] = np.ascontiguousarray(blk)
                off += w
        m["ht"] = ht_blocks
        m["hrow"] = np.ascontiguousarray(
            h[c * TLOC:(c + 1) * TLOC].reshape(JT, 128, D).astype(bf16)
        )
        m["wg"] = np.ascontiguousarray(
            Wg[c * TLOC:(c + 1) * TLOC].reshape(JT, 128, D).astype(bf16)
        )
        in_maps.append(m)

    res = run_bass_kernel_spmd(nc, in_maps, core_ids=list(range(NCORES)))

    # Combine: hsums[c][p, half*TT+t, a] are partial sums of exp(logit)
    # over core c's vocab shard for token t*128+p (unused slots are 0).
    pad = len(CH) * 512 - CPH          # zero-region cols per step
    S = np.stack([r["hsums"] for r in res.results])         # [8,128,2*TT]
    S = S.reshape(NCORES, 128, 2, TT).sum(axis=2)           # [8,128,TT]
    sumexp = S.transpose(0, 2, 1).reshape(NCORES, T).astype(np.float64)
    sumexp -= 2.0 * pad
    logz = np.log(sumexp.sum(axis=0))                       # [T]

    G = np.stack([r["tgt"] for r in res.results])           # [8, 128, JT]
    tgt = G.transpose(0, 2, 1).reshape(T) + tgt_bias        # [T]

    nll = logz - tgt
    lw64 = lw.astype(np.float64)
    loss = (lw64 * nll).sum() / lw64.sum()
    return np.float32(loss)


# revision 9
# speedup vs baseline: 1.0029x; 1.0029x over previous
"""Fused linear + cross-entropy loss (chunked logsumexp) on 8 NeuronCores.

Strategy: tensor-parallel over vocab. Each core holds a 4000-row shard of
head_weight, computes logits = h @ W_c^T for all 4096 tokens (fp8e4m3
DoubleRow matmuls by default; bf16 fallback when head_bias is nonzero),
and reduces sum(exp(logit)) per token on the ACT engine (exp with
accum_out; the pre-exp rescale for the fp8 weight scaling rides the
ACT's free scale operand). The target-logit term is a per-token dot
h[t] . W[label_t] computed on the DVE in bf16 from host-gathered rows
(data-parallel over tokens). The host does only glue: transpose/cast/
shard, the final log over 4096 values, and the weighted mean.
"""

import numpy as np
import ml_dtypes

T = 4096
D = 1024
V = 32000
NCORES = 8
VSH = V // NCORES        # 4000 vocab rows per core
CPH = VSH // 2           # 2000 vocab cols per half
TT = T // 128            # 32 token tiles
TBC = 512                # tokens per resident ht block
NTB = T // TBC           # 8 ht col blocks
TLOC = T // NCORES       # 512 tokens per core for the target dot
JT = TLOC // 128         # 4 local token tiles

W_SCALE = 32.0           # fp8 path: W is scaled by this before casting
USE_FP8 = True

_CACHE = {}


def _chunks(cols):
    """Split cols into matmul free-dim chunks (<=512, 16-aligned)."""
    out = []
    while cols > 0:
        c = min(cols, 512)
        out.append(c)
        cols -= c
    assert all(c % 16 == 0 for c in out)
    return out


def _build(kt, mode, t=T, vsh=VSH, jt=JT, d=D, do_compile=True):
    """Build+compile the SPMD Bass program.

    kt: number of 128-deep k tiles (8, or 9 when a nonzero head_bias is
        folded in as an extra contraction row).
    mode: "bf16" (plain matmuls) or "fp8dr" (fp8e4m3 DoubleRow, kt even).
    """
    import concourse.bass as bass
    import concourse.mybir as mybir
    import concourse.tile as tile
    from concourse import bacc

    f32 = mybir.dt.float32
    bf16 = mybir.dt.bfloat16
    fp8 = mybir.dt.float8e4
    AF = mybir.ActivationFunctionType
    ALU = mybir.AluOpType

    fp8dr = mode == "fp8dr"
    mdt = fp8 if fp8dr else bf16
    act_scale = (1.0 / W_SCALE) if fp8dr else 1.0
    if fp8dr:
        assert kt % 2 == 0
    nk = kt // 2 if fp8dr else kt   # matmul contraction steps

    tt = t // 128
    tb = min(TBC // 128, tt)   # token tiles per ht block
    ntb = tt // tb
    cph = vsh // 2
    CH = _chunks(cph)          # e.g. [512, 512, 512, 464]
    nch = len(CH)
    nsteps = 2 * tt

    nc = bacc.Bacc("TRN2", target_bir_lowering=False, debug=False)

    ht_d = nc.dram_tensor("ht", [ntb, 128, kt, tb * 128], mdt,
                          kind="ExternalInput")
    w_d = {}
    for half in range(2):
        for ci, w in enumerate(CH):
            w_d[half, ci] = nc.dram_tensor(
                f"w_{half}_{ci}", [128, kt, w], mdt, kind="ExternalInput"
            )
    hrow_d = nc.dram_tensor("hrow", [jt, 128, d], bf16, kind="ExternalInput")
    wg_d = nc.dram_tensor("wg", [jt, 128, d], bf16, kind="ExternalInput")
    hsums_d = nc.dram_tensor("hsums", [128, nsteps], f32,
                             kind="ExternalOutput")
    tgt_d = nc.dram_tensor("tgt", [128, jt], f32, kind="ExternalOutput")

    with tile.TileContext(nc) as tc:
        with (
            tc.tile_pool(name="w", bufs=1) as wpool,
            tc.tile_pool(name="h", bufs=1) as hpool,
            tc.tile_pool(name="dot", bufs=1) as dpool,
            tc.tile_pool(name="stat", bufs=1) as spool,
            tc.tile_pool(name="sink", bufs=4) as kpool,
            tc.tile_pool(name="ps", bufs=2, space="PSUM") as ppool,
        ):
            wt = {}
            ht = [None] * ntb

            def load_w(half, ci, split=1):
                w = CH[ci]
                tl = wpool.tile([128, kt, w], mdt, tag=f"w{half}_{ci}")
                if split == 1:
                    nc.sync.dma_start(tl[:], w_d[half, ci][:])
                else:
                    kh = -(-kt // split)
                    for s in range(split):
                        k0, k1 = s * kh, min((s + 1) * kh, kt)
                        nc.sync.dma_start(
                            tl[:, k0:k1, :],
                            w_d[half, ci][:, k0:k1, :],
                        )
                wt[half, ci] = tl

            def load_h(b, split=1):
                tl = hpool.tile([128, kt, tb * 128], mdt, tag=f"h{b}")
                if split == 1:
                    nc.sync.dma_start(tl[:], ht_d[b])
                else:
                    cw = tb * 128 // split
                    for s in range(split):
                        nc.sync.dma_start(
                            tl[:, :, s * cw:(s + 1) * cw],
                            ht_d[b, :, :, s * cw:(s + 1) * cw],
                        )
                ht[b] = tl

            # First-needed data first; compute starts as pieces land.
            load_w(0, 0, split=2)
            load_h(0, split=2)
            for ci in range(1, nch):
                load_w(0, ci)
            for b in range(1, ntb):
                load_h(b)
            for ci in range(nch):
                load_w(1, ci)

            # PE warmup during the DMA wait: junk matmuls from a memset
            # tile keep the HAM activity window busy so real matmuls run
            # at full clock. Writes the first ps slot; real groups clear
            # the bank with start=True before use.
            warm = kpool.tile([128, 256], mdt, tag="warm")
            nc.gpsimd.memset(warm[:], 0.0)
            ps_w = ppool.tile([128, nch, 512], f32, tag="ps")
            for _ in range(44):
                nc.tensor.matmul(
                    ps_w[:, 0, 0:128], warm[:, 0:128], warm[:, 128:256],
                    start=True, stop=True,
                )

            hsums = spool.tile([128, nsteps], f32, tag="hsums")

            # Target dot: tgt[p, j] = sum_d hrow[j,p,d] * wg[j,p,d]  (DVE)
            tgt_sb = spool.tile([128, jt], f32, tag="tgt")
            for j in range(jt):
                hr = dpool.tile([128, d], bf16, tag=f"hr{j}")
                wr = dpool.tile([128, d], bf16, tag=f"wr{j}")
                nc.sync.dma_start(hr[:], hrow_d[j])
                nc.sync.dma_start(wr[:], wg_d[j])
                dsink = kpool.tile([128, d], f32, tag="dsink")
                nc.vector.tensor_tensor(dsink[:], hr[:], wr[:], ALU.mult)
                nc.vector.tensor_reduce(
                    tgt_sb[:, j:j + 1],
                    dsink[:],
                    axis=mybir.AxisListType.X,
                    op=ALU.add,
                )
            nc.sync.dma_start(tgt_d[:], tgt_sb[:])

            def mm(ps, hblk, mlo, half, ki, ci):
                rhs_t = wt[half, ci]
                w = CH[ci]
                if fp8dr:
                    nc.tensor.matmul(
                        ps[:, ci, 0:w],
                        hblk[:, 2 * ki:2 * ki + 2, mlo:mlo + 128],
                        rhs_t[:, 2 * ki:2 * ki + 2, :],
                        start=(ki == 0),
                        stop=(ki == nk - 1),
                        perf_mode=mybir.MatmulPerfMode.DoubleRow,
                    )
                else:
                    nc.tensor.matmul(
                        ps[:, ci, 0:w],
                        hblk[:, ki, mlo:mlo + 128],
                        rhs_t[:, ki, :],
                        start=(ki == 0),
                        stop=(ki == nk - 1),
                    )

            def step(half, t_i, order):
                s = half * tt + t_i
                hblk = ht[t_i // tb]
                mlo = (t_i % tb) * 128
                ps = ppool.tile([128, nch, 512], f32, tag="ps")
                if order == "k":
                    for ki in range(nk):
                        for ci in range(nch):
                            mm(ps, hblk, mlo, half, ki, ci)
                else:
                    for ci in range(nch):
                        for ki in range(nk):
                            mm(ps, hblk, mlo, half, ki, ci)
                # One ACT over all banks. Unwritten PSUM cols (the tail of
                # the last bank) read as zero after start=True cleared the
                # bank, contributing exp(0)=1 each; host subtracts them.
                esink = kpool.tile([128, nch * 512], bf16, tag="esink")
                nc.scalar.activation(
                    esink[:],
                    ps[:, :, :],
                    AF.Exp,
                    scale=act_scale,
                    accum_out=hsums[:, s:s + 1],
                )

            for t_i in range(tt):
                step(0, t_i, "c" if t_i < 4 else "k")
            for t_i in range(tt):
                step(1, t_i, "k")

            nc.sync.dma_start(hsums_d[:], hsums[:])

    if do_compile:
        nc.compile()
    return nc


def _get_nc(kt, mode):
    key = (kt, mode)
    if key not in _CACHE:
        _CACHE[key] = _build(kt, mode)
    return _CACHE[key]


def kernel(hidden_states, head_weight, head_bias, labels, loss_weight):
    from concourse.bass_utils import run_bass_kernel_spmd

    bf16 = ml_dtypes.bfloat16
    fp8 = ml_dtypes.float8_e4m3
    h = np.ascontiguousarray(np.asarray(hidden_states, dtype=np.float32))
    W = np.ascontiguousarray(np.asarray(head_weight, dtype=np.float32))
    b = np.asarray(head_bias, dtype=np.float32)
    lab = np.asarray(labels).astype(np.int64)
    lw = np.asarray(loss_weight, dtype=np.float32)

    use_bias = bool(np.any(b))
    mode = "fp8dr" if (USE_FP8 and not use_bias) else "bf16"
    mdt = fp8 if mode == "fp8dr" else bf16
    wscale = W_SCALE if mode == "fp8dr" else 1.0
    kt = 9 if use_bias else 8
    nc = _get_nc(kt, mode)
    CH = _chunks(CPH)

    # hT[k, p, t] = h[t, k*128+p]; ht blocks [ntb, 128, kt, TBC].
    hT = np.zeros((kt, 128, T), dtype=np.float32)
    hT[:8] = np.ascontiguousarray(h.T).reshape(8, 128, T)
    if use_bias:
        hT[8, 0, :] = 1.0
    ht_blocks = np.ascontiguousarray(
        hT.reshape(kt, 128, NTB, TBC).transpose(2, 1, 0, 3).astype(mdt)
    )

    Wg = W[lab]                     # [T, D] gathered target rows
    tgt_bias = b[lab]               # [T]

    in_maps = []
    for c in range(NCORES):
        Wc = np.ascontiguousarray(W[c * VSH:(c + 1) * VSH].T) * wscale
        # wT[k, p, v] = Wc.T[k*128+p, v] (scaled)
        wT = np.zeros((kt, 128, VSH), dtype=np.float32)
        wT[:8] = Wc.reshape(8, 128, VSH)
        if use_bias:
            wT[8, 0, :] = b[c * VSH:(c + 1) * VSH]
        m = {}
        off = 0
        for half in range(2):
            for ci, w in enumerate(CH):
                blk = wT[:, :, off:off + w].transpose(1, 0, 2).astype(mdt)
                m[f"w_{half}_{ci}"] = np.ascontiguousarray(blk)
                off += w
        m["ht"] = ht_blocks
        m["hrow"] = np.ascontiguousarray(
            h[c * TLOC:(c + 1) * TLOC].reshape(JT, 128, D).astype(bf16)
        )
        m["wg"] = np.ascontiguousarray(
            Wg[c * TLOC:(c + 1) * TLOC].reshape(JT, 128, D).astype(bf16)
        )
        in_maps.append(m)

    res = run_bass_kernel_spmd(nc, in_maps, core_ids=list(range(NCORES)))

    # Combine: hsums[c][p, half*TT+t, a] are partial sums of exp(logit)
    # over core c's vocab shard for token t*128+p (unused slots are 0).
    pad = len(CH) * 512 - CPH          # zero-region cols per step
    S = np.stack([r["hsums"] for r in res.results])         # [8,128,2*TT]
    S = S.reshape(NCORES, 128, 2, TT).sum(axis=2)           # [8,128,TT]
    sumexp = S.transpose(0, 2, 1).reshape(NCORES, T).astype(np.float64)
    sumexp -= 2.0 * pad
    logz = np.log(sumexp.sum(axis=0))                       # [T]

    G = np.stack([r["tgt"] for r in res.results])           # [8, 128, JT]
    tgt = G.transpose(0, 2, 1).reshape(T) + tgt_bias        # [T]

    nll = logz - tgt
    lw64 = lw.astype(np.float64)
    loss = (lw64 * nll).sum() / lw64.sum()
    return np.float32(loss)
